# revision 27
# baseline (speedup 1.0000x reference)
"""Trainium2 Bass kernel for nn_Causal_model_vae (MoE-routed VAE).

Reference computation (N=16384 tokens, DX=DH=1024, S=8 experts):
    mu_h     = leaky(data @ Wm1 + bm1) @ Wm2 + bm2
    logvar_h = leaky(data @ Wv1 + bv1) @ Wv2 + bv2
    h_sample = eps * exp(0.5*logvar_h) + mu_h
    reconstruct[n] = (leaky(h_sample @ We1[s_n] + be1[s_n]) @ We2[s_n] + be2[s_n])
returns (reconstruct, mu_h, logvar_h, h_sample).

Strategy: expert-parallel across the 8 NeuronCores.  The routing ids `s` are
known on the host, so the host sorts tokens by expert, pads each expert's
token list to a common capacity C (= max expert count rounded up to 8), and
core e processes exactly expert e's tokens: the (replicated-weight) encoder
on its C tokens, then ONLY its own expert's decoder — 6 matmul layers per
token instead of the reference's dense 4 + 2*S.

Measured PE cost law on TRN2 (microbenched): a warm bf16 matmul instruction
costs ~2.5ns + moving_cols/2.4GHz (the 128-row stationary load pipelines
fully behind a >=256-col moving phase; the chip sometimes sits in a sticky
2.0GHz power state instead, +19% run-to-run).  Token blocks
[504, 440, 432, 432, 280] for C=2088: a WIDE first block so block 0's
layers consume weights just slower than the (8-core-contended, ~±3us
run-to-run) input chain delivers them, and a narrow LAST block so the
tail after the final matmul (ACT copies + rec DMA + teardown, ~5.7us
floor) is short.  Best measured: 365.9us warm (zero PE stalls, seamless
warmup handoff), vs a ~359us structural floor.

On-chip layout: feature-major [feature, token] so chained matmuls need no
transposes; matmul operands bf16 (f32 PSUM accumulation).  All DRAM tensors
use the flat SBUF-image layout [128, KT*width] (host pre/post-arranges) so
every DMA is fully contiguous per partition.  eps and all four outputs move
as bf16 (rel-err budget 2e-2; bf16 quantization adds <0.4%).

Leaky-relu runs on the ACT engine as Prelu(alpha=0.01) straight out of PSUM
(one op instead of two DVE ops); the sampling chain is
    std = exp(0.5*lv_psum)  [ACT]   tmp = eps*std [DVE]
    h   = tmp + mu_psum     [DVE, reads the wm2 PSUM directly]
so mu never materializes in f32 SBUF.  Encoder layer order is wv1, wv2,
wm1, wm2 so the chain consumes psums as they appear.

Input DMAs are issued on the sync queue in first-use order (FIFO delivery):
x0, then all four encoder weights as 4-m-tile halves, then x1; eps0 issues
from the ACT stream mid-layer-1 (the scalar DMA queue is empty that early,
so it never competes with the weight chain).  Junk warm-up matmuls span the
whole ~19us head: a PE-idle gap >3.4us before the real stream re-throttles
the HAM clock gate and the first ~16 real matmuls run at half clock; a
mid-stream stall >3.4us does the same, which is why the consumption side
is tuned to tolerate the DMA chain's measured worst case.  Output
DMAs issue from the ACT engine (which produced the data, so no cross-engine
wait) and gpsimd (h), keeping the SP chain free to block.  The decoder is
software-pipelined one token-block behind the encoder so the PE never
idles while the sampling chain drains; the final rec block flushes in two
half-DMAs so the first half moves during the last matmuls.

Biases are structurally zero in this problem's setup_inputs(); the kernel
asserts that and falls back to exact host numpy if violated.
"""

import contextlib
import ctypes
import math
import os
import sys
import types

import numpy as np
import ml_dtypes

import concourse.bacc as bacc
import concourse.bass as bass
import concourse.mybir as mybir
import concourse.tile as tile
from concourse.bass_utils import run_bass_kernel_spmd

N, DX, DH, S = 16384, 1024, 1024, 8
KT = DH // 128    # 8 k-tiles (DX == DH == 1024)
SLOPE = 0.01
NCORES = 8
T = 512           # max token block width (one PSUM bank of f32)
WARM = 170        # junk matmuls covering the block-0 input DMA + clock ramp
                  # (span must reach within ~3.4us of the first real MM at
                  # ~19-20us or the HAM re-throttles the PE clock)

BF16 = mybir.dt.bfloat16
F32 = mybir.dt.float32

LAST_RESULTS = None  # BassKernelResults of the most recent run (for profiling)

_program_cache: dict[int, "bacc.Bacc"] = {}


def _ensure_ntff_hook():
    """bass_utils imports antenv.axon_hooks when tracing under axon; some
    images lack that module.  Install a ctypes-based equivalent if so."""
    try:
        import antenv.axon_hooks  # noqa: F401
        return
    except ImportError:
        pass
    try:
        import antenv

        so_path = "/opt/axon/libaxon_pjrt.so"
        if not os.path.exists(so_path):
            return
        lib = ctypes.CDLL(so_path)
        if not hasattr(lib, "axon_start_nrt_profile"):
            return
        lib.axon_start_nrt_profile.argtypes = [
            ctypes.POINTER(ctypes.c_int64), ctypes.c_size_t]
        lib.axon_start_nrt_profile.restype = ctypes.c_int64
        lib.axon_stop_nrt_profile.argtypes = [ctypes.c_char_p]
        lib.axon_stop_nrt_profile.restype = ctypes.c_int64

        @contextlib.contextmanager
        def _hook(output_dir, device_ids):
            import jax

            jax.devices()
            if device_ids:
                ids = (ctypes.c_int64 * len(device_ids))(*device_ids)
                rc = lib.axon_start_nrt_profile(ids, len(device_ids))
            else:
                rc = lib.axon_start_nrt_profile(None, 0)
            if rc != 0:
                raise RuntimeError(f"axon_start_nrt_profile rc={rc}")
            try:
                yield
            finally:
                n = lib.axon_stop_nrt_profile(str(output_dir).encode())
                print(f"ntff profile: {n} file(s) -> {output_dir}")

        m = types.ModuleType("antenv.axon_hooks")
        m.get_axon_ntff_profile_hook = lambda: _hook
        m.set_axon_ntff_profile_hook = lambda h: None
        sys.modules["antenv.axon_hooks"] = m
        antenv.axon_hooks = m
    except Exception:
        pass


def _token_blocks(C):
    """Blocks (multiples of 8, all in [256, 512] when C allows): a ~296-wide
    FIRST block so the x0+wv1-chunk input DMA lands early and the MM stream
    starts sooner, a 256-wide LAST block so the tail (final ACT copies +
    output DMA after the last matmul) is short, 512-wide middles.  Widths
    stay >=256 because below that the 128-col LDWEIGHTS (~107ns) outruns
    the matmul stream and the PE falls to weight-load rate."""
    if C <= 512:
        return [(0, C)]
    # First-block width is tuned against MEASURED 8-core-contended input
    # DMA arrivals (x0 ~2.7us, wv1 halves ~7-8us each, wv2 ~6.2us, ~1us
    # completion->PE semaphore): at 432 every block-0 layer consumes just
    # slower than its weights arrive, so the stream never stalls (a >3.4us
    # stall would also re-throttle the HAM clock and double the cost).
    first, last = 504, 280
    mid_total = C - first - last
    if mid_total <= 0:
        first = (C // 2) // 8 * 8
        return [(0, first), (first, C - first)]
    nmid = max(1, -(-mid_total // 504))
    base = (mid_total // nmid) // 8 * 8
    assert base <= 504
    widths = [base] * nmid
    extra = mid_total - base * nmid
    i = 0
    while extra > 0:
        take = min(8, extra)
        widths[i] += take
        extra -= take
        i = (i + 1) % nmid
    widths = [first] + widths + [last]
    blocks, off = [], 0
    for w in widths:
        blocks.append((off, w))
        off += w
    return blocks


def build_program(C: int) -> "bacc.Bacc":
    assert C % 8 == 0
    blocks = _token_blocks(C)
    nb = len(blocks)

    nc = bacc.Bacc("TRN2", target_bir_lowering=False, debug=False,
                   num_devices=NCORES)

    xT = nc.dram_tensor("xT", [128, KT * C], BF16, kind="ExternalInput").ap()
    epsT = nc.dram_tensor("epsT", [128, KT * C], BF16,
                          kind="ExternalInput").ap()
    # Encoder weights concatenated in first-use order (wv1,wv2,wm1,wm2):
    # ONE dma delivers them sequentially at full queue bandwidth, so each
    # arrives just before its layer needs it with only xT racing the queue.
    wencD = nc.dram_tensor("wenc", [128, 4 * KT * 1024], BF16,
                           kind="ExternalInput").ap()
    wdecD = nc.dram_tensor("wdec", [128, 2 * KT * 1024], BF16,
                           kind="ExternalInput").ap()
    outs = {n: nc.dram_tensor(n, [128, KT * C], BF16,
                              kind="ExternalOutput").ap()
            for n in ["muT", "lvT", "hT", "recT"]}

    Prelu = mybir.ActivationFunctionType.Prelu
    Exp = mybir.ActivationFunctionType.Exp
    Copy = mybir.ActivationFunctionType.Copy
    mult = mybir.AluOpType.mult
    add = mybir.AluOpType.add

    with tile.TileContext(nc) as tc:
        with (
            tc.tile_pool(name="wpool", bufs=1) as wpool,
            tc.tile_pool(name="io2", bufs=3) as io2,
            tc.tile_pool(name="mid", bufs=1) as mid,
            tc.tile_pool(name="hpool", bufs=2) as hpool,
            tc.tile_pool(name="stg", bufs=2) as stg,
            # Separate PSUM pools so the (one-block-delayed) decoder's
            # slot-recycling waits never reference encoder matmul progress.
            tc.tile_pool(name="psum_e", bufs=4,
                         space=bass.MemorySpace.PSUM) as psum_e,
            tc.tile_pool(name="psum_d", bufs=3,
                         space=bass.MemorySpace.PSUM) as psum_d,
            tc.tile_pool(name="psum_w", bufs=1,
                         space=bass.MemorySpace.PSUM) as psum_w,
        ):
            # ---- input DMAs ------------------------------------------
            x_tiles, eps_tiles = {}, {}

            def fetch_x(b, eng):
                off, w = blocks[b]
                t = io2.tile([128, KT * w], BF16, tag="x")
                eng.dma_start(t[:], xT[:, off * KT:off * KT + KT * w])
                x_tiles[b] = t

            def fetch_eps(b, eng):
                off, w = blocks[b]
                t = io2.tile([128, KT * w], BF16, tag="eps")
                eng.dma_start(t[:], epsT[:, off * KT:off * KT + KT * w])
                eps_tiles[b] = t

            # zeros for PE warm-up (memset: no DMA dependency, starts fast)
            z = wpool.tile([128, 128], BF16, tag="z")
            nc.gpsimd.memset(z[:], 0.0)

            # The SP engine's DMA queue is a FIFO striped over the 16 DMA
            # engines (~250ns + bytes/26GB/s per partition-row packet per
            # engine, so ~210-300GB/s for these transfers).  Issue every
            # early-critical input on it as a separate dma_start in exact
            # first-use order: FIFO gives ordered delivery, per-transfer
            # semaphores unblock each layer as its weight lands.  Inputs
            # needed later issue from the ACT stream mid-compute
            # (in-order placement = delayed start, no semaphore chains —
            # those drop the PE clock).
            W1 = KT * 1024
            H1 = W1 // 2
            fetch_x(0, nc.sync)
            wenc = wpool.tile([128, 4 * W1], BF16, tag="wenc")
            # All four encoder weights stream as 4-m-tile halves in
            # first-use order; eps0 is NOT in this chain (it issues from the
            # ACT stream mid-layer-1, see enc_block).  Half-granularity
            # means a late chain only ever stalls the PE by <3.4us (a
            # longer stall would also re-throttle the HAM clock gate and
            # roughly double the damage).
            for ci in range(8):
                nc.sync.dma_start(wenc[:, ci * H1:(ci + 1) * H1],
                                  wencD[:, ci * H1:(ci + 1) * H1])
            if nb > 1:
                fetch_x(1, nc.sync)
            wdec = wpool.tile([128, 2 * W1], BF16, tag="wdec")
            wt = {
                "wv1": (wenc, 0 * KT * 1024),
                "wv2": (wenc, 1 * KT * 1024),
                "wm1": (wenc, 2 * KT * 1024),
                "wm2": (wenc, 3 * KT * 1024),
                "we1": (wdec, 0 * KT * 1024),
                "we2": (wdec, 1 * KT * 1024),
            }

            # ---- PE warm-up ------------------------------------------
            # The first real matmul waits ~14us for x0+wv1; run
            # dependency-free matmuls on the zeroed tile in that window so
            # the HAM clock is at full speed when the real stream starts.
            ps_w = psum_w.tile([128, 256], F32, tag="psw")
            for _ in range(WARM):
                nc.tensor.matmul(ps_w[:, :128], z[:], z[:],
                                 start=True, stop=True)

            # ---- layers ----------------------------------------------
            def layer(w, rhs, tw, out_cb, pool, first_after=None):
                """One 1024->1024 matmul layer on a [128, KT*tw] bf16 rhs.

                Weight image is m-tile-major: (m,k) 128x128 stationary tile
                at cols m*1024 + k*128.  out_cb(mt, ps) consumes the
                [128, tw] f32 psum of m-tile mt.  Returns the last matmul.
                first_after: scheduling hint — place this layer's first
                matmul after that instruction.
                """
                wtile, wb = w
                mm = None
                for mt in range(8):
                    ps = pool.tile([128, tw], F32, tag="ps")
                    for k in range(KT):
                        wo = wb + mt * 1024 + k * 128
                        mm = nc.tensor.matmul(
                            ps[:],
                            wtile[:, wo:wo + 128],
                            rhs[:, k * tw:(k + 1) * tw],
                            start=(k == 0),
                            stop=(k == KT - 1),
                        )
                        if first_after is not None:
                            tile.add_dep_helper(
                                mm.ins, first_after.ins, sync=False,
                                reason="decoder pipelined behind next block")
                            first_after = None
                    out_cb(mt, ps)
                return mm

            def prelu_to(dst, tw):
                def cb(mt, ps):
                    nc.scalar.activation(dst[:, mt * tw:(mt + 1) * tw],
                                         ps[:], Prelu, alpha=SLOPE)
                return cb

            def staged_out(name, off, tw, split=False):
                """ACT copies psum -> a whole-block bf16 staging tile; one
                ACT-issued DMA per layer-block (6-8KB partition rows keep
                the DMA queue at full packet efficiency).  Returns
                (per-m callback, flush).  split=True (for the final rec
                block) flushes in two half-block DMAs so the first half
                moves while the last m-tiles are still in the matmul."""
                s_ = stg.tile([128, KT * tw], BF16, tag="stage")

                def cb(mt, ps):
                    nc.scalar.activation(s_[:, mt * tw:(mt + 1) * tw],
                                         ps[:], Copy)
                    if split and mt == 3:
                        nc.scalar.dma_start(
                            outs[name][:, off * KT:off * KT + 4 * tw],
                            s_[:, :4 * tw])

                def flush():
                    if split:
                        # sync engine is idle at kernel end: it executes the
                        # wait-for-last-copy + DMA issue (~0.7us) while the
                        # ACT stream is still emitting copies.
                        nc.sync.dma_start(
                            outs[name][:, off * KT + 4 * tw:off * KT + 8 * tw],
                            s_[:, 4 * tw:])
                    else:
                        nc.scalar.dma_start(
                            outs[name][:, off * KT:off * KT + KT * tw], s_[:])
                return cb, flush

            def enc_block(b):
                """Encoder + sampling for block b; returns (h_bf16, l1_last)."""
                off, tw = blocks[b]
                x = x_tiles.pop(b)

                h1 = mid.tile([128, KT * tw], BF16, tag="h1")
                base_cb = prelu_to(h1, tw)

                def l1_cb(mt, ps):
                    base_cb(mt, ps)
                    # eps0 issues from the ACT stream right after layer 1's
                    # second prelu: the scalar DMA queue is empty this early,
                    # so eps0 lands ~10us before the wv2-layer DVE needs it
                    # without ever competing with the critical weight chain.
                    if b == 0 and mt == 1:
                        fetch_eps(0, nc.scalar)

                l1_last = layer(wt["wv1"], x, tw, l1_cb, psum_e)

                # ACT has just emitted block b's first prelus: DMAs issued
                # here start only once compute reaches this block — a free
                # ordering mechanism that keeps early bandwidth for x0+wenc.
                if b > 0 and b + 2 < nb:
                    fetch_x(b + 2, nc.scalar)
                    fetch_eps(b + 2, nc.scalar)
                epst = eps_tiles.pop(b)

                std = mid.tile([128, KT * tw], BF16, tag="std")
                tmp = mid.tile([128, KT * tw], BF16, tag="tmp")
                lv_out, lv_flush = staged_out("lvT", off, tw)

                def lv_cb(mt, ps):
                    sl = slice(mt * tw, (mt + 1) * tw)
                    lv_out(mt, ps)
                    nc.scalar.activation(std[:, sl], ps[:], Exp, scale=0.5)
                    nc.vector.tensor_tensor(tmp[:, sl], epst[:, sl],
                                            std[:, sl], mult)

                layer(wt["wv2"], h1, tw, lv_cb, psum_e)
                lv_flush()
                if b == 0:
                    nc.scalar.dma_start(wdec[:, :KT * 1024],
                                        wdecD[:, :KT * 1024])
                    nc.scalar.dma_start(wdec[:, KT * 1024:],
                                        wdecD[:, KT * 1024:])
                    if nb > 1:
                        fetch_eps(1, nc.scalar)

                h1m = mid.tile([128, KT * tw], BF16, tag="h1")
                layer(wt["wm1"], x, tw, prelu_to(h1m, tw), psum_e)
                if b == 0 and nb > 2:
                    fetch_x(2, nc.scalar)
                    fetch_eps(2, nc.scalar)

                h_b = hpool.tile([128, KT * tw], BF16, tag="h")
                mu_out, mu_flush = staged_out("muT", off, tw)

                def mu_cb(mt, ps):
                    sl = slice(mt * tw, (mt + 1) * tw)
                    mu_out(mt, ps)
                    # h = eps*std + mu, straight off the wm2 psum
                    nc.vector.tensor_tensor(h_b[:, sl], tmp[:, sl], ps[:],
                                            add)

                layer(wt["wm2"], h1m, tw, mu_cb, psum_e)
                mu_flush()
                nc.gpsimd.dma_start(outs["hT"][:, off * KT:off * KT + KT * tw],
                                    h_b[:])
                return h_b, l1_last

            def dec_block(b, h_b, first_after=None):
                """Decoder (this core's expert) for block b."""
                off, tw = blocks[b]
                d1 = mid.tile([128, KT * tw], BF16, tag="d1")
                layer(wt["we1"], h_b, tw, prelu_to(d1, tw), psum_d,
                      first_after=first_after)
                rec_out, rec_flush = staged_out("recT", off, tw,
                                                split=(b == nb - 1))
                layer(wt["we2"], d1, tw, rec_out, psum_d)
                rec_flush()

            # Software-pipeline the decoder one block behind the encoder:
            # while block b's sampling chain (ACT exp + DVE fma) drains,
            # the PE is busy on block b-1's decoder.
            prev_h = None
            for b in range(nb):
                h_b, l1_last = enc_block(b)
                if prev_h is not None:
                    dec_block(b - 1, prev_h, first_after=l1_last)
                prev_h = h_b
            dec_block(nb - 1, prev_h)

    nc.compile()
    return nc


def _get_program(C: int) -> "bacc.Bacc":
    if C not in _program_cache:
        _program_cache[C] = build_program(C)
    return _program_cache[C]


def _to_sbuf_image(arrT, blocks):
    """[1024, C] feature-major -> [128, KT*C] flat SBUF image, blockwise."""
    out = np.empty((128, KT * arrT.shape[1]), dtype=arrT.dtype)
    for off, w in blocks:
        seg = arrT[:, off:off + w].reshape(KT, 128, w).transpose(1, 0, 2)
        out[:, off * KT: off * KT + KT * w] = seg.reshape(128, KT * w)
    return out


def _from_sbuf_image(img, blocks, C):
    """[128, KT*C] flat SBUF image -> [1024, C] feature-major."""
    out = np.empty((1024, C), dtype=img.dtype)
    for off, w in blocks:
        seg = img[:, off * KT: off * KT + KT * w].reshape(128, KT, w)
        out[:, off:off + w] = seg.transpose(1, 0, 2).reshape(1024, w)
    return out


def _weight_image(W):
    """[1024 din, 1024 dout] -> [128, KT*1024] flat lhsT image, m-tile-major:
    the (m,k) 128x128 stationary tile sits at cols m*1024 + k*128, so the
    DMA delivers tiles in the exact order the per-m matmul loop needs."""
    img = W.reshape(KT, 128, 8, 128).transpose(1, 2, 0, 3)
    return np.ascontiguousarray(img.reshape(128, KT * 1024))


def _kernel_numpy(inputs):
    """Exact f32 fallback (used only if an assumption is violated)."""
    d = {k: np.asarray(v) for k, v in inputs.items()}
    leaky = lambda v: np.where(v > 0, v, np.float32(SLOPE) * v)
    mu = leaky(d["data"] @ d["Wm1"] + d["bm1"]) @ d["Wm2"] + d["bm2"]
    lv = leaky(d["data"] @ d["Wv1"] + d["bv1"]) @ d["Wv2"] + d["bv2"]
    h = d["eps"] * np.exp(0.5 * lv) + mu
    s = np.asarray(d["s"]).astype(np.int64)
    rec = np.empty_like(d["data"])
    for e in range(d["We1"].shape[0]):
        m = s == e
        rec[m] = leaky(h[m] @ d["We1"][e] + d["be1"][e]) @ d["We2"][e] + d["be2"][e]
    return rec, mu, lv, h


def kernel(**inputs) -> tuple:
    data = np.ascontiguousarray(np.asarray(inputs["data"], dtype=np.float32))
    eps = np.ascontiguousarray(np.asarray(inputs["eps"], dtype=np.float32))
    s = np.asarray(inputs["s"]).astype(np.int64)
    # The device kernel folds the (structurally zero) biases away; any
    # violated assumption falls back to an exact host computation.
    nonzero_bias = any(
        np.abs(np.asarray(inputs[b])).max() != 0.0
        for b in ("bm1", "bm2", "bv1", "bv2", "be1", "be2"))
    if nonzero_bias or data.shape != (N, DX) or s.shape != (N,):
        return _kernel_numpy(inputs)

    counts = np.bincount(s, minlength=S)
    C = max(64, int(math.ceil(counts.max() / 8)) * 8)
    blocks = _token_blocks(C)
    nc = _get_program(C)

    bf = ml_dtypes.bfloat16
    # token ids per expert, padded to C with token 0 (results discarded)
    idx = np.zeros((S, C), dtype=np.int64)
    for e in range(S):
        ids = np.nonzero(s == e)[0]
        idx[e, : len(ids)] = ids

    # encoder weights in first-use order (matches wenc on-device layout)
    wenc = np.concatenate([
        _weight_image(np.asarray(inputs["Wv1"], np.float32)).astype(bf),
        _weight_image(np.asarray(inputs["Wv2"], np.float32)).astype(bf),
        _weight_image(np.asarray(inputs["Wm1"], np.float32)).astype(bf),
        _weight_image(np.asarray(inputs["Wm2"], np.float32)).astype(bf),
    ], axis=1)
    We1 = np.asarray(inputs["We1"], np.float32)
    We2 = np.asarray(inputs["We2"], np.float32)
    dataT = data.T
    epsT = eps.T

    in_maps = []
    for e in range(S):
        ids = idx[e]
        in_maps.append({
            "xT": _to_sbuf_image(
                np.ascontiguousarray(dataT[:, ids]).astype(bf), blocks),
            "epsT": _to_sbuf_image(
                np.ascontiguousarray(epsT[:, ids]).astype(bf), blocks),
            "wenc": wenc,
            "wdec": np.concatenate([
                _weight_image(We1[e]).astype(bf),
                _weight_image(We2[e]).astype(bf)], axis=1),
        })

    global LAST_RESULTS
    _ensure_ntff_hook()
    res = run_bass_kernel_spmd(nc, in_maps, list(range(NCORES)))
    LAST_RESULTS = res

    mu = np.empty((N, DH), np.float32)
    lv = np.empty((N, DH), np.float32)
    h = np.empty((N, DH), np.float32)
    rec = np.empty((N, DX), np.float32)
    for e in range(S):
        cnt = int(counts[e])
        ids = idx[e, :cnt]
        r = res.results[e]
        mu[ids] = _from_sbuf_image(
            np.asarray(r["muT"]).astype(np.float32), blocks, C)[:, :cnt].T
        lv[ids] = _from_sbuf_image(
            np.asarray(r["lvT"]).astype(np.float32), blocks, C)[:, :cnt].T
        h[ids] = _from_sbuf_image(
            np.asarray(r["hT"]).astype(np.float32), blocks, C)[:, :cnt].T
        rec[ids] = _from_sbuf_image(
            np.asarray(r["recT"]).astype(np.float32), blocks, C)[:, :cnt].T
    return rec, mu, lv, h



# revision 29
# speedup vs baseline: 1.0105x; 1.0105x over previous
"""Trainium2 Bass kernel for nn_Causal_model_vae (MoE-routed VAE).

Reference computation (N=16384 tokens, DX=DH=1024, S=8 experts):
    mu_h     = leaky(data @ Wm1 + bm1) @ Wm2 + bm2
    logvar_h = leaky(data @ Wv1 + bv1) @ Wv2 + bv2
    h_sample = eps * exp(0.5*logvar_h) + mu_h
    reconstruct[n] = (leaky(h_sample @ We1[s_n] + be1[s_n]) @ We2[s_n] + be2[s_n])
returns (reconstruct, mu_h, logvar_h, h_sample).

Strategy: expert-parallel across the 8 NeuronCores.  The routing ids `s` are
known on the host, so the host sorts tokens by expert, pads each expert's
token list to a common capacity C (= max expert count rounded up to 8), and
core e processes exactly expert e's tokens: the (replicated-weight) encoder
on its C tokens, then ONLY its own expert's decoder — 6 matmul layers per
token instead of the reference's dense 4 + 2*S.

Measured PE cost law on TRN2 (microbenched): a warm bf16 matmul instruction
costs ~2.5ns + moving_cols/2.4GHz (the 128-row stationary load pipelines
fully behind a >=256-col moving phase; the chip sometimes sits in a sticky
2.0GHz power state instead, +19% run-to-run).  Token blocks
[504, 440, 432, 432, 280] for C=2088: a WIDE first block so block 0's
layers consume weights just slower than the (8-core-contended, ~±3us
run-to-run) input chain delivers them, and a narrow LAST block so the
tail after the final matmul (ACT copies + rec DMA + teardown, ~5.7us
floor) is short.  Best measured: 365.9us warm (zero PE stalls, seamless
warmup handoff), vs a ~359us structural floor.

On-chip layout: feature-major [feature, token] so chained matmuls need no
transposes; matmul operands bf16 (f32 PSUM accumulation).  All DRAM tensors
use the flat SBUF-image layout [128, KT*width] (host pre/post-arranges) so
every DMA is fully contiguous per partition.  eps and all four outputs move
as bf16 (rel-err budget 2e-2; bf16 quantization adds <0.4%).

Leaky-relu runs on the ACT engine as Prelu(alpha=0.01) straight out of PSUM
(one op instead of two DVE ops); the sampling chain is
    std = exp(0.5*lv_psum)  [ACT]   tmp = eps*std [DVE]
    h   = tmp + mu_psum     [DVE, reads the wm2 PSUM directly]
so mu never materializes in f32 SBUF.  Encoder layer order is wv1, wv2,
wm1, wm2 so the chain consumes psums as they appear.

Input DMAs are issued on the sync queue in first-use order (FIFO delivery):
x0, then all four encoder weights as 4-m-tile halves, then x1; eps0 issues
from the ACT stream mid-layer-1 (the scalar DMA queue is empty that early,
so it never competes with the weight chain).  Junk warm-up matmuls span the
whole ~19us head: a PE-idle gap >3.4us before the real stream re-throttles
the HAM clock gate and the first ~16 real matmuls run at half clock; a
mid-stream stall >3.4us does the same, which is why the consumption side
is tuned to tolerate the DMA chain's measured worst case.  Output
DMAs issue from the ACT engine (which produced the data, so no cross-engine
wait) and gpsimd (h), keeping the SP chain free to block.  The decoder is
software-pipelined one token-block behind the encoder so the PE never
idles while the sampling chain drains; the final rec block flushes in two
half-DMAs so the first half moves during the last matmuls.

Biases are structurally zero in this problem's setup_inputs(); the kernel
asserts that and falls back to exact host numpy if violated.
"""

import contextlib
import ctypes
import math
import os
import sys
import types

import numpy as np
import ml_dtypes

import concourse.bacc as bacc
import concourse.bass as bass
import concourse.mybir as mybir
import concourse.tile as tile
from concourse.bass_utils import run_bass_kernel_spmd

N, DX, DH, S = 16384, 1024, 1024, 8
KT = DH // 128    # 8 k-tiles (DX == DH == 1024)
SLOPE = 0.01
NCORES = 8
T = 512           # max token block width (one PSUM bank of f32)
WARM = 70         # junk matmuls covering the block-0 input DMA + clock ramp
                  # (span must reach within ~3.4us of the first real MM at
                  # ~14-15us or the HAM re-throttles the PE clock)

BF16 = mybir.dt.bfloat16
F32 = mybir.dt.float32

LAST_RESULTS = None  # BassKernelResults of the most recent run (for profiling)

_program_cache: dict[int, "bacc.Bacc"] = {}


def _ensure_ntff_hook():
    """bass_utils imports antenv.axon_hooks when tracing under axon; some
    images lack that module.  Install a ctypes-based equivalent if so."""
    try:
        import antenv.axon_hooks  # noqa: F401
        return
    except ImportError:
        pass
    try:
        import antenv

        so_path = "/opt/axon/libaxon_pjrt.so"
        if not os.path.exists(so_path):
            return
        lib = ctypes.CDLL(so_path)
        if not hasattr(lib, "axon_start_nrt_profile"):
            return
        lib.axon_start_nrt_profile.argtypes = [
            ctypes.POINTER(ctypes.c_int64), ctypes.c_size_t]
        lib.axon_start_nrt_profile.restype = ctypes.c_int64
        lib.axon_stop_nrt_profile.argtypes = [ctypes.c_char_p]
        lib.axon_stop_nrt_profile.restype = ctypes.c_int64

        @contextlib.contextmanager
        def _hook(output_dir, device_ids):
            import jax

            jax.devices()
            if device_ids:
                ids = (ctypes.c_int64 * len(device_ids))(*device_ids)
                rc = lib.axon_start_nrt_profile(ids, len(device_ids))
            else:
                rc = lib.axon_start_nrt_profile(None, 0)
            if rc != 0:
                raise RuntimeError(f"axon_start_nrt_profile rc={rc}")
            try:
                yield
            finally:
                n = lib.axon_stop_nrt_profile(str(output_dir).encode())
                print(f"ntff profile: {n} file(s) -> {output_dir}")

        m = types.ModuleType("antenv.axon_hooks")
        m.get_axon_ntff_profile_hook = lambda: _hook
        m.set_axon_ntff_profile_hook = lambda h: None
        sys.modules["antenv.axon_hooks"] = m
        antenv.axon_hooks = m
    except Exception:
        pass


def _token_blocks(C):
    """Blocks (multiples of 8, all in [256, 512] when C allows): a ~296-wide
    FIRST block so the x0+wv1-chunk input DMA lands early and the MM stream
    starts sooner, a 256-wide LAST block so the tail (final ACT copies +
    output DMA after the last matmul) is short, 512-wide middles.  Widths
    stay >=256 because below that the 128-col LDWEIGHTS (~107ns) outruns
    the matmul stream and the PE falls to weight-load rate."""
    if C <= 512:
        return [(0, C)]
    # First-block width is tuned against MEASURED 8-core-contended input
    # DMA arrivals (x0 ~2.7us, wv1 halves ~7-8us each, wv2 ~6.2us, ~1us
    # completion->PE semaphore): at 432 every block-0 layer consumes just
    # slower than its weights arrive, so the stream never stalls (a >3.4us
    # stall would also re-throttle the HAM clock and double the cost).
    first, last = 504, 280
    mid_total = C - first - last
    if mid_total <= 0:
        first = (C // 2) // 8 * 8
        return [(0, first), (first, C - first)]
    nmid = max(1, -(-mid_total // 504))
    base = (mid_total // nmid) // 8 * 8
    assert base <= 504
    widths = [base] * nmid
    extra = mid_total - base * nmid
    i = 0
    while extra > 0:
        take = min(8, extra)
        widths[i] += take
        extra -= take
        i = (i + 1) % nmid
    widths = [first] + widths + [last]
    blocks, off = [], 0
    for w in widths:
        blocks.append((off, w))
        off += w
    return blocks


def build_program(C: int) -> "bacc.Bacc":
    assert C % 8 == 0
    blocks = _token_blocks(C)
    nb = len(blocks)

    nc = bacc.Bacc("TRN2", target_bir_lowering=False, debug=False,
                   num_devices=NCORES)

    xT = nc.dram_tensor("xT", [128, KT * C], BF16, kind="ExternalInput").ap()
    epsT = nc.dram_tensor("epsT", [128, KT * C], BF16,
                          kind="ExternalInput").ap()
    # Encoder weights concatenated in first-use order (wv1,wv2,wm1,wm2):
    # ONE dma delivers them sequentially at full queue bandwidth, so each
    # arrives just before its layer needs it with only xT racing the queue.
    wencD = nc.dram_tensor("wenc", [128, 4 * KT * 1024], BF16,
                           kind="ExternalInput").ap()
    wdecD = nc.dram_tensor("wdec", [128, 2 * KT * 1024], BF16,
                           kind="ExternalInput").ap()
    outs = {n: nc.dram_tensor(n, [128, KT * C], BF16,
                              kind="ExternalOutput").ap()
            for n in ["muT", "lvT", "hT", "recT"]}

    Prelu = mybir.ActivationFunctionType.Prelu
    Exp = mybir.ActivationFunctionType.Exp
    Copy = mybir.ActivationFunctionType.Copy
    mult = mybir.AluOpType.mult
    add = mybir.AluOpType.add

    with tile.TileContext(nc) as tc:
        with (
            tc.tile_pool(name="wpool", bufs=1) as wpool,
            tc.tile_pool(name="io2", bufs=3) as io2,
            tc.tile_pool(name="mid", bufs=1) as mid,
            tc.tile_pool(name="hpool", bufs=2) as hpool,
            tc.tile_pool(name="stg", bufs=2) as stg,
            # Separate PSUM pools so the (one-block-delayed) decoder's
            # slot-recycling waits never reference encoder matmul progress.
            tc.tile_pool(name="psum_e", bufs=4,
                         space=bass.MemorySpace.PSUM) as psum_e,
            tc.tile_pool(name="psum_d", bufs=3,
                         space=bass.MemorySpace.PSUM) as psum_d,
            tc.tile_pool(name="psum_w", bufs=1,
                         space=bass.MemorySpace.PSUM) as psum_w,
        ):
            # ---- input DMAs ------------------------------------------
            x_tiles, eps_tiles = {}, {}

            def fetch_x(b, eng):
                off, w = blocks[b]
                t = io2.tile([128, KT * w], BF16, tag="x")
                eng.dma_start(t[:], xT[:, off * KT:off * KT + KT * w])
                x_tiles[b] = t

            def fetch_eps(b, eng):
                off, w = blocks[b]
                t = io2.tile([128, KT * w], BF16, tag="eps")
                eng.dma_start(t[:], epsT[:, off * KT:off * KT + KT * w])
                eps_tiles[b] = t

            # zeros for PE warm-up (memset: no DMA dependency, starts fast)
            z = wpool.tile([128, 128], BF16, tag="z")
            nc.gpsimd.memset(z[:], 0.0)

            # The SP engine's DMA queue is a FIFO striped over the 16 DMA
            # engines (~250ns + bytes/26GB/s per partition-row packet per
            # engine, so ~210-300GB/s for these transfers).  Issue every
            # early-critical input on it as a separate dma_start in exact
            # first-use order: FIFO gives ordered delivery, per-transfer
            # semaphores unblock each layer as its weight lands.  Inputs
            # needed later issue from the ACT stream mid-compute
            # (in-order placement = delayed start, no semaphore chains —
            # those drop the PE clock).
            W1 = KT * 1024
            H1 = W1 // 2
            fetch_x(0, nc.sync)
            wenc = wpool.tile([128, 4 * W1], BF16, tag="wenc")
            # wv1 streams as eight 256KB single-m-tile chunks: at 504-wide
            # block 0 each m-tile takes ~1.7us to consume vs ~1.2-2us to
            # deliver, so the chunks stay just ahead AND the first real
            # matmul only gates on x0 + one chunk (~14us vs ~19-28us).
            # wv2/wm1/wm2 stream as 4-m-tile halves; eps0 is NOT in this
            # chain (it issues from the ACT stream mid-layer-1, see
            # enc_block).  Chunk granularity keeps any late-chain PE stall
            # well under 3.4us — a longer stall would also re-throttle the
            # HAM clock gate and roughly double the damage.
            Q1 = W1 // 8
            for ci in range(8):
                nc.sync.dma_start(wenc[:, ci * Q1:(ci + 1) * Q1],
                                  wencD[:, ci * Q1:(ci + 1) * Q1])
            for ci in range(2, 8):
                nc.sync.dma_start(wenc[:, ci * H1:(ci + 1) * H1],
                                  wencD[:, ci * H1:(ci + 1) * H1])
            if nb > 1:
                fetch_x(1, nc.sync)
            wdec = wpool.tile([128, 2 * W1], BF16, tag="wdec")
            wt = {
                "wv1": (wenc, 0 * KT * 1024),
                "wv2": (wenc, 1 * KT * 1024),
                "wm1": (wenc, 2 * KT * 1024),
                "wm2": (wenc, 3 * KT * 1024),
                "we1": (wdec, 0 * KT * 1024),
                "we2": (wdec, 1 * KT * 1024),
            }

            # ---- PE warm-up ------------------------------------------
            # The first real matmul waits ~14us for x0+wv1; run
            # dependency-free matmuls on the zeroed tile in that window so
            # the HAM clock is at full speed when the real stream starts.
            ps_w = psum_w.tile([128, 256], F32, tag="psw")
            for _ in range(WARM):
                nc.tensor.matmul(ps_w[:, :128], z[:], z[:],
                                 start=True, stop=True)

            # ---- layers ----------------------------------------------
            def layer(w, rhs, tw, out_cb, pool, first_after=None):
                """One 1024->1024 matmul layer on a [128, KT*tw] bf16 rhs.

                Weight image is m-tile-major: (m,k) 128x128 stationary tile
                at cols m*1024 + k*128.  out_cb(mt, ps) consumes the
                [128, tw] f32 psum of m-tile mt.  Returns the last matmul.
                first_after: scheduling hint — place this layer's first
                matmul after that instruction.
                """
                wtile, wb = w
                mm = None
                for mt in range(8):
                    ps = pool.tile([128, tw], F32, tag="ps")
                    for k in range(KT):
                        wo = wb + mt * 1024 + k * 128
                        mm = nc.tensor.matmul(
                            ps[:],
                            wtile[:, wo:wo + 128],
                            rhs[:, k * tw:(k + 1) * tw],
                            start=(k == 0),
                            stop=(k == KT - 1),
                        )
                        if first_after is not None:
                            tile.add_dep_helper(
                                mm.ins, first_after.ins, sync=False,
                                reason="decoder pipelined behind next block")
                            first_after = None
                    out_cb(mt, ps)
                return mm

            def prelu_to(dst, tw):
                def cb(mt, ps):
                    nc.scalar.activation(dst[:, mt * tw:(mt + 1) * tw],
                                         ps[:], Prelu, alpha=SLOPE)
                return cb

            def staged_out(name, off, tw, split=False):
                """ACT copies psum -> a whole-block bf16 staging tile; one
                ACT-issued DMA per layer-block (6-8KB partition rows keep
                the DMA queue at full packet efficiency).  Returns
                (per-m callback, flush).  split=True (for the final rec
                block) flushes in two half-block DMAs so the first half
                moves while the last m-tiles are still in the matmul."""
                s_ = stg.tile([128, KT * tw], BF16, tag="stage")

                def cb(mt, ps):
                    nc.scalar.activation(s_[:, mt * tw:(mt + 1) * tw],
                                         ps[:], Copy)
                    if split and mt == 3:
                        nc.scalar.dma_start(
                            outs[name][:, off * KT:off * KT + 4 * tw],
                            s_[:, :4 * tw])

                def flush():
                    if split:
                        # sync engine is idle at kernel end: it executes the
                        # wait-for-last-copy + DMA issue (~0.7us) while the
                        # ACT stream is still emitting copies.
                        nc.sync.dma_start(
                            outs[name][:, off * KT + 4 * tw:off * KT + 8 * tw],
                            s_[:, 4 * tw:])
                    else:
                        nc.scalar.dma_start(
                            outs[name][:, off * KT:off * KT + KT * tw], s_[:])
                return cb, flush

            def enc_block(b):
                """Encoder + sampling for block b; returns (h_bf16, l1_last)."""
                off, tw = blocks[b]
                x = x_tiles.pop(b)

                h1 = mid.tile([128, KT * tw], BF16, tag="h1")
                base_cb = prelu_to(h1, tw)

                def l1_cb(mt, ps):
                    base_cb(mt, ps)
                    # eps0 issues from the ACT stream right after layer 1's
                    # second prelu: the scalar DMA queue is empty this early,
                    # so eps0 lands ~10us before the wv2-layer DVE needs it
                    # without ever competing with the critical weight chain.
                    if b == 0 and mt == 1:
                        fetch_eps(0, nc.scalar)

                l1_last = layer(wt["wv1"], x, tw, l1_cb, psum_e)

                # ACT has just emitted block b's first prelus: DMAs issued
                # here start only once compute reaches this block — a free
                # ordering mechanism that keeps early bandwidth for x0+wenc.
                if b > 0 and b + 2 < nb:
                    fetch_x(b + 2, nc.scalar)
                    fetch_eps(b + 2, nc.scalar)
                epst = eps_tiles.pop(b)

                std = mid.tile([128, KT * tw], BF16, tag="std")
                tmp = mid.tile([128, KT * tw], BF16, tag="tmp")
                lv_out, lv_flush = staged_out("lvT", off, tw)

                def lv_cb(mt, ps):
                    sl = slice(mt * tw, (mt + 1) * tw)
                    lv_out(mt, ps)
                    nc.scalar.activation(std[:, sl], ps[:], Exp, scale=0.5)
                    nc.vector.tensor_tensor(tmp[:, sl], epst[:, sl],
                                            std[:, sl], mult)

                layer(wt["wv2"], h1, tw, lv_cb, psum_e)
                lv_flush()
                if b == 0:
                    nc.scalar.dma_start(wdec[:, :KT * 1024],
                                        wdecD[:, :KT * 1024])
                    nc.scalar.dma_start(wdec[:, KT * 1024:],
                                        wdecD[:, KT * 1024:])
                    if nb > 1:
                        fetch_eps(1, nc.scalar)

                h1m = mid.tile([128, KT * tw], BF16, tag="h1")
                layer(wt["wm1"], x, tw, prelu_to(h1m, tw), psum_e)
                if b == 0 and nb > 2:
                    fetch_x(2, nc.scalar)
                    fetch_eps(2, nc.scalar)

                h_b = hpool.tile([128, KT * tw], BF16, tag="h")
                mu_out, mu_flush = staged_out("muT", off, tw)

                def mu_cb(mt, ps):
                    sl = slice(mt * tw, (mt + 1) * tw)
                    mu_out(mt, ps)
                    # h = eps*std + mu, straight off the wm2 psum
                    nc.vector.tensor_tensor(h_b[:, sl], tmp[:, sl], ps[:],
                                            add)

                layer(wt["wm2"], h1m, tw, mu_cb, psum_e)
                mu_flush()
                nc.gpsimd.dma_start(outs["hT"][:, off * KT:off * KT + KT * tw],
                                    h_b[:])
                return h_b, l1_last

            def dec_block(b, h_b, first_after=None):
                """Decoder (this core's expert) for block b."""
                off, tw = blocks[b]
                d1 = mid.tile([128, KT * tw], BF16, tag="d1")
                layer(wt["we1"], h_b, tw, prelu_to(d1, tw), psum_d,
                      first_after=first_after)
                rec_out, rec_flush = staged_out("recT", off, tw,
                                                split=(b == nb - 1))
                layer(wt["we2"], d1, tw, rec_out, psum_d)
                rec_flush()

            # Software-pipeline the decoder one block behind the encoder:
            # while block b's sampling chain (ACT exp + DVE fma) drains,
            # the PE is busy on block b-1's decoder.
            prev_h = None
            for b in range(nb):
                h_b, l1_last = enc_block(b)
                if prev_h is not None:
                    dec_block(b - 1, prev_h, first_after=l1_last)
                prev_h = h_b
            dec_block(nb - 1, prev_h)

    nc.compile()
    return nc


def _get_program(C: int) -> "bacc.Bacc":
    if C not in _program_cache:
        _program_cache[C] = build_program(C)
    return _program_cache[C]


def _to_sbuf_image(arrT, blocks):
    """[1024, C] feature-major -> [128, KT*C] flat SBUF image, blockwise."""
    out = np.empty((128, KT * arrT.shape[1]), dtype=arrT.dtype)
    for off, w in blocks:
        seg = arrT[:, off:off + w].reshape(KT, 128, w).transpose(1, 0, 2)
        out[:, off * KT: off * KT + KT * w] = seg.reshape(128, KT * w)
    return out


def _from_sbuf_image(img, blocks, C):
    """[128, KT*C] flat SBUF image -> [1024, C] feature-major."""
    out = np.empty((1024, C), dtype=img.dtype)
    for off, w in blocks:
        seg = img[:, off * KT: off * KT + KT * w].reshape(128, KT, w)
        out[:, off:off + w] = seg.transpose(1, 0, 2).reshape(1024, w)
    return out


def _weight_image(W):
    """[1024 din, 1024 dout] -> [128, KT*1024] flat lhsT image, m-tile-major:
    the (m,k) 128x128 stationary tile sits at cols m*1024 + k*128, so the
    DMA delivers tiles in the exact order the per-m matmul loop needs."""
    img = W.reshape(KT, 128, 8, 128).transpose(1, 2, 0, 3)
    return np.ascontiguousarray(img.reshape(128, KT * 1024))


def _kernel_numpy(inputs):
    """Exact f32 fallback (used only if an assumption is violated)."""
    d = {k: np.asarray(v) for k, v in inputs.items()}
    leaky = lambda v: np.where(v > 0, v, np.float32(SLOPE) * v)
    mu = leaky(d["data"] @ d["Wm1"] + d["bm1"]) @ d["Wm2"] + d["bm2"]
    lv = leaky(d["data"] @ d["Wv1"] + d["bv1"]) @ d["Wv2"] + d["bv2"]
    h = d["eps"] * np.exp(0.5 * lv) + mu
    s = np.asarray(d["s"]).astype(np.int64)
    rec = np.empty_like(d["data"])
    for e in range(d["We1"].shape[0]):
        m = s == e
        rec[m] = leaky(h[m] @ d["We1"][e] + d["be1"][e]) @ d["We2"][e] + d["be2"][e]
    return rec, mu, lv, h


def kernel(**inputs) -> tuple:
    data = np.ascontiguousarray(np.asarray(inputs["data"], dtype=np.float32))
    eps = np.ascontiguousarray(np.asarray(inputs["eps"], dtype=np.float32))
    s = np.asarray(inputs["s"]).astype(np.int64)
    # The device kernel folds the (structurally zero) biases away; any
    # violated assumption falls back to an exact host computation.
    nonzero_bias = any(
        np.abs(np.asarray(inputs[b])).max() != 0.0
        for b in ("bm1", "bm2", "bv1", "bv2", "be1", "be2"))
    if nonzero_bias or data.shape != (N, DX) or s.shape != (N,):
        return _kernel_numpy(inputs)

    counts = np.bincount(s, minlength=S)
    C = max(64, int(math.ceil(counts.max() / 8)) * 8)
    blocks = _token_blocks(C)
    nc = _get_program(C)

    bf = ml_dtypes.bfloat16
    # token ids per expert, padded to C with token 0 (results discarded)
    idx = np.zeros((S, C), dtype=np.int64)
    for e in range(S):
        ids = np.nonzero(s == e)[0]
        idx[e, : len(ids)] = ids

    # encoder weights in first-use order (matches wenc on-device layout)
    wenc = np.concatenate([
        _weight_image(np.asarray(inputs["Wv1"], np.float32)).astype(bf),
        _weight_image(np.asarray(inputs["Wv2"], np.float32)).astype(bf),
        _weight_image(np.asarray(inputs["Wm1"], np.float32)).astype(bf),
        _weight_image(np.asarray(inputs["Wm2"], np.float32)).astype(bf),
    ], axis=1)
    We1 = np.asarray(inputs["We1"], np.float32)
    We2 = np.asarray(inputs["We2"], np.float32)
    dataT = data.T
    epsT = eps.T

    in_maps = []
    for e in range(S):
        ids = idx[e]
        in_maps.append({
            "xT": _to_sbuf_image(
                np.ascontiguousarray(dataT[:, ids]).astype(bf), blocks),
            "epsT": _to_sbuf_image(
                np.ascontiguousarray(epsT[:, ids]).astype(bf), blocks),
            "wenc": wenc,
            "wdec": np.concatenate([
                _weight_image(We1[e]).astype(bf),
                _weight_image(We2[e]).astype(bf)], axis=1),
        })

    global LAST_RESULTS
    _ensure_ntff_hook()
    res = run_bass_kernel_spmd(nc, in_maps, list(range(NCORES)))
    LAST_RESULTS = res

    mu = np.empty((N, DH), np.float32)
    lv = np.empty((N, DH), np.float32)
    h = np.empty((N, DH), np.float32)
    rec = np.empty((N, DX), np.float32)
    for e in range(S):
        cnt = int(counts[e])
        ids = idx[e, :cnt]
        r = res.results[e]
        mu[ids] = _from_sbuf_image(
            np.asarray(r["muT"]).astype(np.float32), blocks, C)[:, :cnt].T
        lv[ids] = _from_sbuf_image(
            np.asarray(r["lvT"]).astype(np.float32), blocks, C)[:, :cnt].T
        h[ids] = _from_sbuf_image(
            np.asarray(r["hT"]).astype(np.float32), blocks, C)[:, :cnt].T
        rec[ids] = _from_sbuf_image(
            np.asarray(r["recT"]).astype(np.float32), blocks, C)[:, :cnt].T
    return rec, mu, lv, h

